# revision 1
# baseline (speedup 1.0000x reference)
"""Trainium2 Bass kernel for nn_Block_80736795230695 (AFNO block).

Math (see reference):
  n1 = LN1(x); attn = AFNO(n1) + n1; x2 = x + attn
  n2 = LN2(x2); h = n2 @ fc1 + b; h = dwconv3x3(h); h = gelu(h)
  out = x2 + h @ fc2 + b2

Decomposition (validated on host in numpy to ~1e-6 rel):
  - rfft2/irfft2 become brute-force DFT matmuls over the 1024 spatial
    positions (H=W=32); mode packing m = r*544 + kh*17 + kw.
  - LN1 gamma folds into AFNO layer-1 weight rows; LN1 beta only affects
    the DC mode and becomes a [96]-vector added to mode column 0.
  - LN2 gamma/beta fold into fc1.
  - Depthwise 3x3 conv on a guard-padded 34x34 grid: center tap +
    conv-bias initialize PSUM on ScalarE, 4 taps are diagonal matmuls on
    TensorE, 4 taps are fused scalar_tensor_tensor MACs on VectorE.

Sharding: data-parallel over batch: 16 batches -> 8 cores x 2.
"""

import numpy as np
import ml_dtypes

DIM = 384
NUM_BLOCKS = 8
BS = 48
HID = 1536
H = W = 32
WF = 17
NMODE = H * WF  # 544
LAM = 0.01
EPS = 1e-5
B_LOC = 2
N_CORES = 8

TAP_ACT = (0, 0)
TAPS_PE = [(-1, 0), (1, 0), (0, -1), (0, 1)]
TAPS_DVE = [(-1, -1), (-1, 1), (1, -1), (1, 1)]

PADW = 34
HP_LEN = 2 + PADW * PADW  # 1 guard + 34x34 + 1 guard = 1158
# conv output covers padded positions p in [34, 1122); psum idx = p - 34;
# h_pad read index for tap offset d is 1 + p + d (>= 0 because d >= -35).
CONV_LEN = 1088
CONV_WINS = ((0, 512), (512, 512), (1024, 64))
IDFT_CHUNKS = [128, 128, 128, 128, 32]


def _f32(a):
    return np.ascontiguousarray(np.asarray(a), dtype=np.float32)


def _bf16(a):
    return np.ascontiguousarray(np.asarray(a, np.float32).astype(ml_dtypes.bfloat16))


def host_derived(inputs):
    ln1_g = np.asarray(inputs["ln1_g"], np.float64)
    ln1_b = np.asarray(inputs["ln1_b"], np.float64)
    ln2_g = np.asarray(inputs["ln2_g"], np.float64)
    ln2_b = np.asarray(inputs["ln2_b"], np.float64)
    w1 = np.asarray(inputs["afno_w1"], np.float64)
    b1 = np.asarray(inputs["afno_b1"], np.float64)
    w2 = np.asarray(inputs["afno_w2"], np.float64)
    b2 = np.asarray(inputs["afno_b2"], np.float64)
    fc1_w = np.asarray(inputs["fc1_w"], np.float64)
    fc1_b = np.asarray(inputs["fc1_b"], np.float64)
    dw_w = np.asarray(inputs["dw_w"], np.float64)
    dw_b = np.asarray(inputs["dw_b"], np.float64)
    fc2_w = np.asarray(inputs["fc2_w"], np.float64)
    fc2_b = np.asarray(inputs["fc2_b"], np.float64)

    # forward DFT [1024, 1088]
    hh, ww = np.meshgrid(np.arange(H), np.arange(W), indexing="ij")
    khs = np.arange(H)[:, None, None, None]
    kws = np.arange(WF)[None, :, None, None]
    ang = -2 * np.pi * (khs * hh[None, None] / H + kws * ww[None, None] / W)
    sc = 1.0 / np.sqrt(H * W)
    tf = np.zeros((H * W, 2 * NMODE))
    tf[:, :NMODE] = (np.cos(ang) * sc).reshape(NMODE, H * W).T
    tf[:, NMODE:] = (np.sin(ang) * sc).reshape(NMODE, H * W).T

    # inverse DFT with Hermitian fold, rows ordered by chunk q = t*2 + r
    d = np.where((np.arange(WF) == 0) | (np.arange(WF) == WF - 1), 1.0, 2.0)
    are = (np.cos(-ang) * d[None, :, None, None]) * sc
    aim = (-np.sin(-ang) * d[None, :, None, None]) * sc
    a_inv = np.concatenate(
        [are.reshape(NMODE, H * W), aim.reshape(NMODE, H * W)], axis=0)
    a_ord = np.zeros_like(a_inv)
    off = 0
    row0 = 0
    for rows in IDFT_CHUNKS:
        for r in range(2):
            a_ord[off:off + rows] = a_inv[r * NMODE + row0: r * NMODE + row0 + rows]
            off += rows
        row0 += rows

    # AFNO layer 1 with LN1 folds; zero-padded block-diagonal lhsT halves
    gblk = ln1_g.reshape(NUM_BLOCKS, BS, 1)
    w1r_f = w1[0] * gblk
    w1i_f = w1[1] * gblk
    dc_r = float(H) * np.einsum("ni,nio->no", ln1_b.reshape(NUM_BLOCKS, BS), w1[0])
    dc_i = float(H) * np.einsum("ni,nio->no", ln1_b.reshape(NUM_BLOCKS, BS), w1[1])
    w1c_a = np.zeros((96, NUM_BLOCKS * 96))
    w1c_b = np.zeros((96, NUM_BLOCKS * 96))
    b1c = np.zeros((96, NUM_BLOCKS))
    dcv = np.zeros((96, NUM_BLOCKS))
    for n in range(NUM_BLOCKS):
        r0 = (n % 2) * BS
        w1c_a[r0:r0 + BS, n * 96: n * 96 + BS] = w1r_f[n]
        w1c_a[r0:r0 + BS, n * 96 + BS: n * 96 + 96] = w1i_f[n]
        w1c_b[r0:r0 + BS, n * 96: n * 96 + BS] = -w1i_f[n]
        w1c_b[r0:r0 + BS, n * 96 + BS: n * 96 + 96] = w1r_f[n]
        b1c[:BS, n] = b1[0][n]
        b1c[BS:, n] = b1[1][n]
        dcv[:BS, n] = dc_r[n]
        dcv[BS:, n] = dc_i[n]

    # AFNO layer 2: lhsT is the o1 tile (rows 0:48 re, 48:96 im, 96 ones);
    # rhs [97, 96] per block, output cols (o, r) interleaved.
    w2c = np.zeros((97, NUM_BLOCKS * 96))
    for n in range(NUM_BLOCKS):
        blk = np.zeros((97, 96))
        blk[0:BS, 0::2] = w2[0][n]
        blk[0:BS, 1::2] = w2[1][n]
        blk[BS:96, 0::2] = -w2[1][n]
        blk[BS:96, 1::2] = w2[0][n]
        blk[96, 0::2] = b2[0][n]
        blk[96, 1::2] = b2[1][n]
        w2c[:, n * 96: (n + 1) * 96] = blk

    w1f = fc1_w * ln2_g[:, None]
    b1f = fc1_b + ln2_b @ fc1_w

    dwk = dw_w[:, 0]  # [1536, 3, 3]

    def tapv(dy, dx):
        return dwk[:, dy + 1, dx + 1].reshape(12, 128).T

    dvev = np.stack([tapv(dy, dx) for (dy, dx) in TAPS_DVE], axis=1)  # [128,4,12]
    ddiag = np.zeros((12, 128, 4 * 128))
    for t in range(12):
        for j, (dy, dx) in enumerate(TAPS_PE):
            np.fill_diagonal(ddiag[t, :, j * 128: (j + 1) * 128],
                             dwk[t * 128: (t + 1) * 128, dy + 1, dx + 1])

    return {
        "tf": _bf16(tf),
        "ainv": _bf16(a_ord),
        "w1ca": _bf16(w1c_a),
        "w1cb": _bf16(w1c_b),
        "b1c": _f32(b1c),
        "dcv": _f32(dcv),
        "w2c": _bf16(w2c),
        "ln1g": _f32(inputs["ln1_g"]),
        "ln1b": _f32(inputs["ln1_b"]),
        "w1f": _bf16(w1f),
        "b1fv": _f32(b1f.reshape(12, 128).T),
        "w2f": _bf16(fc2_w),
        "b2f": _bf16(fc2_b.reshape(1, DIM)),
        "w0v": _f32(tapv(*TAP_ACT)),
        "dwbv": _f32(dw_b.reshape(12, 128).T),
        "dvev": _f32(dvev.reshape(128, 48)),  # col = j*12 + t
        "ddiag": _bf16(ddiag),
        "onesr": _bf16(np.ones((1, NMODE))),
        "zpad": _bf16(np.zeros((128, HP_LEN))),
    }


def build_nc():
    import concourse.bass as bass
    import concourse.bacc as bacc
    import concourse.mybir as mybir
    import concourse.tile as tile
    from concourse.masks import make_identity
    from contextlib import ExitStack

    f32 = mybir.dt.float32
    bf16 = mybir.dt.bfloat16
    ALU = mybir.AluOpType
    ACTF = mybir.ActivationFunctionType

    nc = bacc.Bacc("TRN2", target_bir_lowering=False, debug=False,
                   num_devices=N_CORES)

    xd = nc.dram_tensor("x", [B_LOC, 1024, DIM], f32, kind="ExternalInput")
    yd = nc.dram_tensor("y", [B_LOC, 1024, DIM], f32, kind="ExternalOutput")
    dx2d = nc.dram_tensor("dbg_x2", [8, 128, B_LOC, DIM], f32, kind="ExternalOutput")
    dhpd = nc.dram_tensor("dbg_hp", [12, 128, HP_LEN], bf16, kind="ExternalOutput")
    dyd = nc.dram_tensor("dbg_y", [12, 128, 1024], bf16, kind="ExternalOutput")
    tfd = nc.dram_tensor("tf", [1024, 2 * NMODE], bf16, kind="ExternalInput")
    aid = nc.dram_tensor("ainv", [2 * NMODE, 1024], bf16, kind="ExternalInput")
    w1cad = nc.dram_tensor("w1ca", [96, 768], bf16, kind="ExternalInput")
    w1cbd = nc.dram_tensor("w1cb", [96, 768], bf16, kind="ExternalInput")
    b1cd = nc.dram_tensor("b1c", [96, 8], f32, kind="ExternalInput")
    dcvd = nc.dram_tensor("dcv", [96, 8], f32, kind="ExternalInput")
    w2cd = nc.dram_tensor("w2c", [97, 768], bf16, kind="ExternalInput")
    ln1gd = nc.dram_tensor("ln1g", [DIM], f32, kind="ExternalInput")
    ln1bd = nc.dram_tensor("ln1b", [DIM], f32, kind="ExternalInput")
    w1fd = nc.dram_tensor("w1f", [DIM, HID], bf16, kind="ExternalInput")
    b1fvd = nc.dram_tensor("b1fv", [128, 12], f32, kind="ExternalInput")
    w2fd = nc.dram_tensor("w2f", [HID, DIM], bf16, kind="ExternalInput")
    b2fd = nc.dram_tensor("b2f", [1, DIM], bf16, kind="ExternalInput")
    w0vd = nc.dram_tensor("w0v", [128, 12], f32, kind="ExternalInput")
    dwbvd = nc.dram_tensor("dwbv", [128, 12], f32, kind="ExternalInput")
    dvevd = nc.dram_tensor("dvev", [128, 48], f32, kind="ExternalInput")
    ddiagd = nc.dram_tensor("ddiag", [12, 128, 512], bf16, kind="ExternalInput")
    onesrd = nc.dram_tensor("onesr", [1, NMODE], bf16, kind="ExternalInput")
    zpadd = nc.dram_tensor("zpad", [128, HP_LEN], bf16, kind="ExternalInput")

    def shifted(ap2d, elem_off, length):
        return bass.AP(tensor=ap2d.tensor, offset=ap2d.offset + elem_off,
                       ap=[ap2d.ap[0], [1, length]])

    with tile.TileContext(nc) as tc, \
         ExitStack() as ctx:
        dma = nc.sync.dma_start

        keep = ctx.enter_context(tc.tile_pool(name="keep", bufs=1))
        stat = ctx.enter_context(tc.tile_pool(name="stat", bufs=8))
        x2_pool = ctx.enter_context(tc.tile_pool(name="x2p", bufs=1))
        eps_t = keep.tile([128, 1], f32, tag="eps")
        nc.vector.memset(eps_t[:, :], EPS)

        def layer_stats(xs, tagsuf):
            st = stat.tile([128, 6], f32, tag="st" + tagsuf)
            nc.vector.bn_stats(out=st[:, :], in_=xs)
            mv = stat.tile([128, 2], f32, tag="mv" + tagsuf)
            nc.vector.bn_aggr(out=mv[:, :], in_=st[:, :])
            rstd = stat.tile([128, 1], f32, tag="rs" + tagsuf)
            nc.scalar.activation(out=rstd[:, :], in_=mv[:, 1:2],
                                 func=ACTF.Sqrt, bias=eps_t[:, :])
            nc.vector.reciprocal(out=rstd[:, :], in_=rstd[:, :])
            return mv, rstd

        # ---------------- Phase A: load x, LN1 ----------------
        # Pool alloc order is load-bearing: releases must be LIFO.
        x_pool = tc.alloc_tile_pool(name="xp", bufs=1)
        n1_pool = tc.alloc_tile_pool(name="n1p", bufs=1)
        xf_pool = tc.alloc_tile_pool(name="xfp", bufs=1)
        z1_pool = tc.alloc_tile_pool(name="z1p", bufs=1)
        bc_pool = tc.alloc_tile_pool(name="bcp", bufs=1)

        g_bc = bc_pool.tile([128, B_LOC, DIM], f32, tag="g_bc")
        b_bc = bc_pool.tile([128, B_LOC, DIM], f32, tag="b_bc")
        gp = ln1gd.ap()
        bp = ln1bd.ap()
        dma(out=g_bc[:, :, :],
            in_=bass.AP(tensor=gp.tensor, offset=0,
                        ap=[[0, 128], [0, B_LOC], [1, DIM]]))
        dma(out=b_bc[:, :, :],
            in_=bass.AP(tensor=bp.tensor, offset=0,
                        ap=[[0, 128], [0, B_LOC], [1, DIM]]))

        xr = xd.ap().rearrange("b (n p) c -> n p b c", p=128)
        x_t, z1_t, n1_t = [], [], []
        with nc.named_scope("ln1"):
            for i in range(8):
                xt = x_pool.tile([128, B_LOC, DIM], f32, tag=f"x{i}")
                dma(out=xt[:, :, :], in_=xr[i])
                x_t.append(xt)
            for i in range(8):
                z1 = z1_pool.tile([128, B_LOC, DIM], bf16, tag=f"z1_{i}")
                n1 = n1_pool.tile([128, B_LOC, DIM], f32, tag=f"n1_{i}")
                for b in range(B_LOC):
                    xs = x_t[i][:, b, :]
                    mv, rstd = layer_stats(xs, "1")
                    nc.vector.tensor_scalar(out=z1[:, b, :], in0=xs,
                                            scalar1=mv[:, 0:1],
                                            scalar2=rstd[:, :],
                                            op0=ALU.subtract, op1=ALU.mult)
                nc.vector.tensor_mul(out=n1[:, :, :], in0=z1[:, :, :],
                                     in1=g_bc[:, :, :])
                nc.vector.tensor_add(out=n1[:, :, :], in0=n1[:, :, :],
                                     in1=b_bc[:, :, :])
                z1_t.append(z1)
                n1_t.append(n1)
        bc_pool.release()

        # ---------------- Phase B: forward DFT ----------------
        xf_t = []
        with nc.named_scope("dft"):
            with tc.tile_pool(name="tfp", bufs=1) as tf_pool, \
                 tc.tile_pool(name="dftps", bufs=4, space="PSUM") as dft_ps:
                tft = []
                for k in range(8):
                    t = tf_pool.tile([128, 2 * NMODE], bf16, tag=f"tf{k}")
                    dma(out=t[:, :], in_=tfd.ap()[128 * k: 128 * (k + 1), :])
                    tft.append(t)
                for mt in range(8):
                    b, bpair = divmod(mt, 4)
                    xf = xf_pool.tile([96, 2 * NMODE], bf16, tag=f"xf{mt}")
                    for c0, clen in ((0, 512), (512, 512), (576, 512)):
                        ps = dft_ps.tile([96, 512], f32, tag="dftps")
                        for k in range(8):
                            nc.tensor.matmul(
                                ps[:, :clen],
                                z1_t[k][:, b, 96 * bpair: 96 * bpair + 96],
                                tft[k][:, c0: c0 + clen],
                                start=(k == 0), stop=(k == 7))
                        nc.scalar.copy(out=xf[:, c0: c0 + clen],
                                       in_=ps[:, :clen])
                    xf_t.append(xf)
        z1_pool.release()

        # ---------------- Phase C: frequency-domain block MLP ----------------
        o2s_pool = tc.alloc_tile_pool(name="o2sp", bufs=1)
        o2s_t = [o2s_pool.tile([IDFT_CHUNKS[t], B_LOC, DIM, 2], bf16,
                               tag=f"o2s{t}", name=f"o2s{t}") for t in range(5)]
        with nc.named_scope("blockmlp"):
            with tc.tile_pool(name="afw", bufs=1) as afw, \
                 tc.tile_pool(name="o1p", bufs=3) as o1_pool, \
                 tc.tile_pool(name="l1ps", bufs=4, space="PSUM") as l1_ps, \
                 tc.tile_pool(name="l2ps", bufs=4, space="PSUM") as l2_ps:
                w1ca = afw.tile([96, 768], bf16, tag="w1ca")
                w1cb = afw.tile([96, 768], bf16, tag="w1cb")
                w2c = afw.tile([97, 768], bf16, tag="w2c")
                b1c = afw.tile([96, 8], f32, tag="b1c")
                dcv = afw.tile([96, 8], f32, tag="dcv")
                dma(out=w1ca[:, :], in_=w1cad.ap())
                dma(out=w1cb[:, :], in_=w1cbd.ap())
                dma(out=w2c[:, :], in_=w2cd.ap())
                dma(out=b1c[:, :], in_=b1cd.ap())
                dma(out=dcv[:, :], in_=dcvd.ap())
                for b in range(B_LOC):
                    for n in range(NUM_BLOCKS):
                        xf = xf_t[b * 4 + n // 2]
                        o1 = o1_pool.tile([97, NMODE], bf16, tag="o1")
                        dma(out=o1[96:97, :], in_=onesrd.ap())
                        for chk in range(2):
                            ps = l1_ps.tile([96, 272], f32, tag="l1ps")
                            csl = slice(272 * chk, 272 * (chk + 1))
                            nc.tensor.matmul(
                                ps[:, :],
                                w1ca[:, 96 * n: 96 * (n + 1)],
                                xf[:, csl],
                                start=True, stop=False)
                            nc.tensor.matmul(
                                ps[:, :],
                                w1cb[:, 96 * n: 96 * (n + 1)],
                                xf[:, NMODE + 272 * chk:
                                   NMODE + 272 * (chk + 1)],
                                start=False, stop=True)
                            if chk == 0:
                                nc.vector.tensor_add(out=ps[:, 0:1],
                                                     in0=ps[:, 0:1],
                                                     in1=dcv[:, n: n + 1])
                            nc.scalar.activation(out=o1[0:96, csl],
                                                 in_=ps[:, :], func=ACTF.Relu,
                                                 bias=b1c[:, n: n + 1])
                        for mc in range(5):
                            rows = IDFT_CHUNKS[mc]
                            ps2 = l2_ps.tile([128, 96], f32, tag="l2ps")
                            nc.tensor.matmul(
                                ps2[:rows, :],
                                o1[:, 128 * mc: 128 * mc + rows],
                                w2c[:, 96 * n: 96 * (n + 1)],
                                start=True, stop=True)
                            nc.scalar.copy(
                                out=o2s_t[mc][:rows, b, n * BS: (n + 1) * BS, :]
                                    .rearrange("p c r -> p (c r)"),
                                in_=ps2[:rows, :])

        # softshrink in place
        with nc.named_scope("softshrink"):
            with tc.tile_pool(name="sshp", bufs=2) as ssh_pool:
                for t in range(5):
                    rows = IDFT_CHUNKS[t]
                    flat = o2s_t[t][:, :, :, :].rearrange("p b c r -> p (b c r)")
                    tmp = ssh_pool.tile([128, B_LOC * DIM * 2], bf16, tag="ssh")
                    nc.vector.tensor_scalar(out=tmp[:rows, :], in0=flat,
                                            scalar1=-LAM, scalar2=LAM,
                                            op0=ALU.max, op1=ALU.min)
                    nc.vector.scalar_tensor_tensor(
                        out=flat, in0=tmp[:rows, :], scalar=-1.0, in1=flat,
                        op0=ALU.mult, op1=ALU.add)

        # ---------------- Phase D: inverse DFT + residual ----------------
        x2_t = []
        with nc.named_scope("idft"):
            for i in range(8):
                nc.vector.tensor_add(out=x_t[i][:, :, :], in0=x_t[i][:, :, :],
                                     in1=n1_t[i][:, :, :])
            with tc.tile_pool(name="aip", bufs=1) as ai_pool, \
                 tc.tile_pool(name="idftps", bufs=4, space="PSUM") as idft_ps:
                av = []
                aoff = 0
                for q in range(10):
                    rows = IDFT_CHUNKS[q // 2]
                    t = ai_pool.tile([128, 1024], bf16, tag=f"av{q}")
                    dma(out=t[:rows, :], in_=aid.ap()[aoff: aoff + rows, :])
                    av.append(t)
                    aoff += rows
                for mt in range(8):
                    x2 = x2_pool.tile([128, B_LOC, DIM], f32, tag=f"x2_{mt}")
                    for b in range(B_LOC):
                        ps = idft_ps.tile([128, DIM], f32, tag="idftps")
                        for q in range(10):
                            rows = IDFT_CHUNKS[q // 2]
                            nc.tensor.matmul(
                                ps[:, :],
                                av[q][:rows, 128 * mt: 128 * (mt + 1)],
                                o2s_t[q // 2][:rows, b, :, q % 2],
                                start=(q == 0), stop=(q == 9))
                        nc.vector.tensor_add(out=x2[:, b, :],
                                             in0=x_t[mt][:, b, :], in1=ps[:, :])
                    dma(out=dx2d.ap()[mt], in_=x2[:, :, :])
                    x2_t.append(x2)
        o2s_pool.release()
        xf_pool.release()
        n1_pool.release()
        x_pool.release()

        # ---------------- Phase E: MLP ----------------
        mlpw = ctx.enter_context(tc.tile_pool(name="mlpw", bufs=1))
        w1ft = []
        for k in range(3):
            t = mlpw.tile([128, HID], bf16, tag=f"w1f{k}")
            dma(out=t[:, :], in_=w1fd.ap()[128 * k: 128 * (k + 1), :])
            w1ft.append(t)
        w2ft = []
        for k in range(12):
            t = mlpw.tile([128, DIM], bf16, tag=f"w2f{k}")
            dma(out=t[:, :], in_=w2fd.ap()[128 * k: 128 * (k + 1), :])
            w2ft.append(t)
        b2f = mlpw.tile([1, DIM], bf16, tag="b2f")
        dma(out=b2f[:, :], in_=b2fd.ap())
        ones_b = mlpw.tile([1, 1024], bf16, tag="ones")
        nc.vector.memset(ones_b[:, :], 1.0)
        ident = mlpw.tile([128, 128], bf16, tag="ident")
        make_identity(nc, ident[:, :])
        b1fv = mlpw.tile([128, 12], f32, tag="b1fv")
        w0v = mlpw.tile([128, 12], f32, tag="w0v")
        dwbv = mlpw.tile([128, 12], f32, tag="dwbv")
        dvev = mlpw.tile([128, 48], f32, tag="dvev")
        dma(out=b1fv[:, :], in_=b1fvd.ap())
        dma(out=w0v[:, :], in_=w0vd.ap())
        dma(out=dwbv[:, :], in_=dwbvd.ap())
        dma(out=dvev[:, :], in_=dvevd.ap())
        ddiag_t = []
        for t in range(12):
            dt_ = mlpw.tile([128, 512], bf16, tag=f"dd{t}")
            dma(out=dt_[:, :], in_=ddiagd.ap()[t])
            ddiag_t.append(dt_)
        hp_t = []
        y_t = []
        for t in range(12):
            hp = mlpw.tile([128, HP_LEN], bf16, tag=f"hp{t}")
            dma(out=hp[:, :], in_=zpadd.ap())
            hp_t.append(hp)
            y_t.append(mlpw.tile([128, 1024], bf16, tag=f"y{t}", name=f"y{t}"))
        z2T_t = [mlpw.tile([128, 1024], bf16, tag=f"z2T{c}", name=f"z2T{c}") for c in range(3)]

        with tc.tile_pool(name="z2p", bufs=3) as z2_pool, \
             tc.tile_pool(name="outp", bufs=3) as out_pool:
            for b in range(B_LOC):
                ln2_ctx = tc.tile_pool(name="trps", bufs=2, space="PSUM")
                tr_ps = ln2_ctx.__enter__()
                fc1_ctx = tc.tile_pool(name="fc1ps", bufs=4, space="PSUM")
                fc1_ps = fc1_ctx.__enter__()
                with nc.named_scope("ln2t"):
                    for i in range(8):
                        xs = x2_t[i][:, b, :]
                        z2 = z2_pool.tile([128, DIM], bf16, tag="z2")
                        mv, rstd = layer_stats(xs, "2")
                        nc.vector.tensor_scalar(out=z2[:, :], in0=xs,
                                                scalar1=mv[:, 0:1],
                                                scalar2=rstd[:, :],
                                                op0=ALU.subtract, op1=ALU.mult)
                        for c in range(3):
                            pst = tr_ps.tile([128, 128], bf16, tag="trps")
                            nc.tensor.transpose(
                                pst[:, :], z2[:, 128 * c: 128 * (c + 1)],
                                ident[:, :])
                            nc.scalar.copy(
                                out=z2T_t[c][:, 128 * i: 128 * (i + 1)],
                                in_=pst[:, :])
                with nc.named_scope("fc1"):
                    for t in range(12):
                        for ncb in range(2):
                            ps = fc1_ps.tile([128, 512], f32, tag="fc1ps")
                            for k in range(3):
                                nc.tensor.matmul(
                                    ps[:, :],
                                    w1ft[k][:, 128 * t: 128 * (t + 1)],
                                    z2T_t[k][:, 512 * ncb: 512 * (ncb + 1)],
                                    start=(k == 0), stop=(k == 2))
                            hp2d = hp_t[t][:, :]
                            dest = bass.AP(
                                tensor=hp2d.tensor,
                                offset=hp2d.offset + 36 + 544 * ncb,
                                ap=[hp2d.ap[0], [34, 16], [1, 32]])
                            nc.scalar.activation(
                                out=dest,
                                in_=ps[:, :].rearrange("p (h w) -> p h w", w=32),
                                func=ACTF.Identity, bias=b1fv[:, t: t + 1])
                if b == 0:
                    for t in range(12):
                        dma(out=dhpd.ap()[t], in_=hp_t[t][:, :])
                fc1_ctx.__exit__(None, None, None)
                ln2_ctx.__exit__(None, None, None)
                conv_ctx = tc.tile_pool(name="convps", bufs=2, space="PSUM")
                conv_ps = conv_ctx.__enter__()
                fc2_ctx = tc.tile_pool(name="fc2ps", bufs=2, space="PSUM")
                fc2_ps = fc2_ctx.__enter__()
                with nc.named_scope("conv"):
                    for t in range(12):
                        hp2d = hp_t[t][:, :]
                        cps = conv_ps.tile([128, CONV_LEN], f32, tag="convps")
                        nc.scalar.activation(
                            out=cps[:, :], in_=hp_t[t][:, 35: 35 + CONV_LEN],
                            func=ACTF.Identity, bias=dwbv[:, t: t + 1],
                            scale=w0v[:, t: t + 1])
                        for (w0, wlen) in CONV_WINS:
                            for j, (dy, dx) in enumerate(TAPS_PE):
                                dd = dy * PADW + dx
                                nc.tensor.matmul(
                                    cps[:, w0: w0 + wlen],
                                    ddiag_t[t][:, 128 * j: 128 * (j + 1)],
                                    shifted(hp2d, 35 + w0 + dd, wlen),
                                    start=False, stop=(j == len(TAPS_PE) - 1),
                                    skip_group_check=True)
                        for j, (dy, dx) in enumerate(TAPS_DVE):
                            dd = dy * PADW + dx
                            nc.vector.scalar_tensor_tensor(
                                out=cps[:, :], in0=shifted(hp2d, 35 + dd, CONV_LEN),
                                scalar=dvev[:, 12 * j + t: 12 * j + t + 1],
                                in1=cps[:, :], op0=ALU.mult, op1=ALU.add)
                        cps2d = cps[:, :]
                        nc.scalar.activation(
                            out=y_t[t][:, :].rearrange("p (h w) -> p h w", w=32),
                            in_=bass.AP(tensor=cps2d.tensor,
                                        offset=cps2d.offset + 1,
                                        ap=[cps2d.ap[0], [34, 32], [1, 32]]),
                            func=ACTF.Gelu)
                with nc.named_scope("fc2"):
                    for mc in range(8):
                        ps = fc2_ps.tile([128, DIM], f32, tag="fc2ps")
                        for k in range(12):
                            nc.tensor.matmul(
                                ps[:, :], y_t[k][:, 128 * mc: 128 * (mc + 1)],
                                w2ft[k][:, :], start=(k == 0), stop=False)
                        nc.tensor.matmul(
                            ps[:, :], ones_b[:, 128 * mc: 128 * (mc + 1)],
                            b2f[:, :], start=False, stop=True)
                        ot = out_pool.tile([128, DIM], f32, tag="out")
                        nc.vector.tensor_add(out=ot[:, :],
                                             in0=x2_t[mc][:, b, :], in1=ps[:, :])
                        dma(out=yd.ap()[b, 128 * mc: 128 * (mc + 1), :],
                            in_=ot[:, :])
                if b == 0:
                    for t in range(12):
                        dma(out=dyd.ap()[t], in_=y_t[t][:, :])
                fc2_ctx.__exit__(None, None, None)
                conv_ctx.__exit__(None, None, None)

    return nc


_NC_CACHE = None


def kernel(**inputs):
    global _NC_CACHE
    from concourse.bass_utils import run_bass_kernel_spmd

    x = np.ascontiguousarray(np.asarray(inputs["x"], np.float32))
    assert int(inputs["H"]) == H and int(inputs["W"]) == W
    der = host_derived(inputs)

    if _NC_CACHE is None:
        nc = build_nc()
        nc.compile()
        _NC_CACHE = nc
    nc = _NC_CACHE

    in_maps = []
    for c in range(N_CORES):
        m = dict(der)
        m["x"] = np.ascontiguousarray(x[c * B_LOC: (c + 1) * B_LOC])
        in_maps.append(m)
    res = run_bass_kernel_spmd(nc, in_maps, core_ids=list(range(N_CORES)))
    out = np.concatenate([res.results[c]["y"] for c in range(N_CORES)], axis=0)
    return out.astype(np.float32)



# revision 12
# speedup vs baseline: 1.4773x; 1.4773x over previous
"""Trainium2 Bass kernel for nn_Block_80736795230695 (AFNO block).

Math (see reference):
  n1 = LN1(x); attn = AFNO(n1) + n1; x2 = x + attn
  n2 = LN2(x2); h = n2 @ fc1 + b; h = dwconv3x3(h); h = gelu(h)
  out = x2 + h @ fc2 + b2

Decomposition (validated on host in numpy to ~1e-6 rel):
  - rfft2/irfft2 become brute-force DFT matmuls over the 1024 spatial
    positions (H=W=32); mode packing m = r*544 + kh*17 + kw.
  - LN1 gamma folds into AFNO layer-1 weight rows; LN1 beta only affects
    the DC mode and becomes a [96]-vector added to mode column 0.
  - LN2 gamma/beta fold into fc1.
  - Depthwise 3x3 conv on a guard-padded 34x34 grid: 4 rect taps are
    diagonal matmuls on TensorE (PSUM); center + 2 diag taps on VectorE
    and 2 diag taps on GpSimd accumulate in a bf16 SBUF tile which is
    injected into PSUM via an identity matmul.
  - Residuals (z1 into x2, x2 into out) are injected into PSUM with
    identity matmuls instead of vector adds.

Sharding: data-parallel over batch: 16 batches -> 8 cores x 2.
"""

import numpy as np
import ml_dtypes

DIM = 384
NUM_BLOCKS = 8
BS = 48
HID = 1536
H = W = 32
WF = 17
NMODE = H * WF  # 544
LAM = 0.01
EPS = 1e-5
B_LOC = 2
N_CORES = 8

TAP_ACT = (0, 0)
TAPS_PE = [(-1, 0), (1, 0), (0, -1), (0, 1)]
TAPS_DVE = [(-1, -1), (-1, 1), (1, -1), (1, 1)]

PADW = 34
HP_LEN = 2 + PADW * PADW  # 1 guard + 34x34 + 1 guard = 1158
# conv output covers padded positions p in [34, 1122); psum idx = p - 34;
# h_pad read index for tap offset d is 1 + p + d (>= 0 because d >= -35).
CONV_LEN = 1088
CONV_HALF = 544
IDFT_CHUNKS = [128, 128, 128, 128, 32]


def _f32(a):
    return np.ascontiguousarray(np.asarray(a), dtype=np.float32)


def _bf16(a):
    return np.ascontiguousarray(np.asarray(a, np.float32).astype(ml_dtypes.bfloat16))


def host_derived(inputs):
    ln1_g = np.asarray(inputs["ln1_g"], np.float64)
    ln1_b = np.asarray(inputs["ln1_b"], np.float64)
    ln2_g = np.asarray(inputs["ln2_g"], np.float64)
    ln2_b = np.asarray(inputs["ln2_b"], np.float64)
    w1 = np.asarray(inputs["afno_w1"], np.float64)
    b1 = np.asarray(inputs["afno_b1"], np.float64)
    w2 = np.asarray(inputs["afno_w2"], np.float64)
    b2 = np.asarray(inputs["afno_b2"], np.float64)
    fc1_w = np.asarray(inputs["fc1_w"], np.float64)
    fc1_b = np.asarray(inputs["fc1_b"], np.float64)
    dw_w = np.asarray(inputs["dw_w"], np.float64)
    dw_b = np.asarray(inputs["dw_b"], np.float64)
    fc2_w = np.asarray(inputs["fc2_w"], np.float64)
    fc2_b = np.asarray(inputs["fc2_b"], np.float64)

    # forward DFT [1024, 1088]
    hh, ww = np.meshgrid(np.arange(H), np.arange(W), indexing="ij")
    khs = np.arange(H)[:, None, None, None]
    kws = np.arange(WF)[None, :, None, None]
    ang = -2 * np.pi * (khs * hh[None, None] / H + kws * ww[None, None] / W)
    sc = 1.0 / np.sqrt(H * W)
    tf = np.zeros((H * W, 2 * NMODE))
    tf[:, :NMODE] = (np.cos(ang) * sc).reshape(NMODE, H * W).T
    tf[:, NMODE:] = (np.sin(ang) * sc).reshape(NMODE, H * W).T

    # inverse DFT with Hermitian fold, rows ordered by chunk q = t*2 + r
    d = np.where((np.arange(WF) == 0) | (np.arange(WF) == WF - 1), 1.0, 2.0)
    are = (np.cos(-ang) * d[None, :, None, None]) * sc
    aim = (-np.sin(-ang) * d[None, :, None, None]) * sc
    a_inv = np.concatenate(
        [are.reshape(NMODE, H * W), aim.reshape(NMODE, H * W)], axis=0)
    a_ord = np.zeros_like(a_inv)
    off = 0
    row0 = 0
    for rows in IDFT_CHUNKS:
        for r in range(2):
            a_ord[off:off + rows] = a_inv[r * NMODE + row0: r * NMODE + row0 + rows]
            off += rows
        row0 += rows

    # AFNO layer 1 with LN1 folds; zero-padded block-diagonal lhsT halves
    gblk = ln1_g.reshape(NUM_BLOCKS, BS, 1)
    w1r_f = w1[0] * gblk
    w1i_f = w1[1] * gblk
    dc_r = float(H) * np.einsum("ni,nio->no", ln1_b.reshape(NUM_BLOCKS, BS), w1[0])
    dc_i = float(H) * np.einsum("ni,nio->no", ln1_b.reshape(NUM_BLOCKS, BS), w1[1])
    w1c_a = np.zeros((96, NUM_BLOCKS * 96))
    w1c_b = np.zeros((96, NUM_BLOCKS * 96))
    b1c = np.zeros((96, NUM_BLOCKS))
    dcv = np.zeros((96, NUM_BLOCKS))
    for n in range(NUM_BLOCKS):
        r0 = (n % 2) * BS
        w1c_a[r0:r0 + BS, n * 96: n * 96 + BS] = w1r_f[n]
        w1c_a[r0:r0 + BS, n * 96 + BS: n * 96 + 96] = w1i_f[n]
        w1c_b[r0:r0 + BS, n * 96: n * 96 + BS] = -w1i_f[n]
        w1c_b[r0:r0 + BS, n * 96 + BS: n * 96 + 96] = w1r_f[n]
        b1c[:BS, n] = b1[0][n]
        b1c[BS:, n] = b1[1][n]
        dcv[:BS, n] = dc_r[n]
        dcv[BS:, n] = dc_i[n]

    # AFNO layer 2: lhsT is the o1 tile (rows 0:48 re, 48:96 im, 96 ones);
    # rhs [97, 96] per block, output cols (o, r) interleaved.
    w2c = np.zeros((97, NUM_BLOCKS * 96))
    for n in range(NUM_BLOCKS):
        blk = np.zeros((97, 96))
        blk[0:BS, 0::2] = w2[0][n]
        blk[0:BS, 1::2] = w2[1][n]
        blk[BS:96, 0::2] = -w2[1][n]
        blk[BS:96, 1::2] = w2[0][n]
        blk[96, 0::2] = b2[0][n]
        blk[96, 1::2] = b2[1][n]
        w2c[:, n * 96: (n + 1) * 96] = blk

    w1f = fc1_w * ln2_g[:, None]
    b1f = fc1_b + ln2_b @ fc1_w

    dwk = dw_w[:, 0]  # [1536, 3, 3]

    def tapv(dy, dx):
        return dwk[:, dy + 1, dx + 1].reshape(12, 128).T

    dvev = np.stack([tapv(dy, dx) for (dy, dx) in TAPS_DVE],
                    axis=1)  # [128, 4, 12]
    ddiag = np.zeros((12, 128, 4 * 128))
    for t in range(12):
        for j, (dy, dx) in enumerate(TAPS_PE):
            np.fill_diagonal(ddiag[t, :, j * 128: (j + 1) * 128],
                             dwk[t * 128: (t + 1) * 128, dy + 1, dx + 1])

    return {
        "tf": _bf16(tf),
        "ainv": _bf16(a_ord),
        "w1ca": _bf16(w1c_a),
        "w1cb": _bf16(w1c_b),
        "b1c": _f32(b1c),
        "dcv": _f32(dcv),
        "w2c": _bf16(w2c),
        "ln1g": _bf16(np.broadcast_to(ln1_g[None, :], (128, DIM))),
        "ln1b": _bf16(np.broadcast_to(ln1_b[None, :], (128, DIM))),
        "w1f": _bf16(w1f),
        "b1fv": _f32(b1f.reshape(12, 128).T),
        "w2f": _bf16(fc2_w),
        "b2f": _bf16(fc2_b.reshape(1, DIM)),
        "w0v": _f32(tapv(*TAP_ACT)),
        "dwbv": _f32(dw_b.reshape(12, 128).T),
        "dvev": _f32(dvev.reshape(128, 48)),  # col = j*12 + t
        "ddiag": _bf16(ddiag),
    }


def build_nc(ln1_trivial=True):
    import concourse.bass as bass
    import concourse.bacc as bacc
    import concourse.mybir as mybir
    import concourse.tile as tile
    from concourse.masks import make_identity
    from contextlib import ExitStack

    f32 = mybir.dt.float32
    bf16 = mybir.dt.bfloat16
    f16 = mybir.dt.float16
    ALU = mybir.AluOpType
    ACTF = mybir.ActivationFunctionType

    nc = bacc.Bacc("TRN2", target_bir_lowering=False, debug=False,
                   num_devices=N_CORES)

    xd = nc.dram_tensor("x", [B_LOC, 1024, DIM], f32, kind="ExternalInput")
    yd = nc.dram_tensor("y", [B_LOC, 1024, DIM], f32, kind="ExternalOutput")
    tfd = nc.dram_tensor("tf", [1024, 2 * NMODE], bf16, kind="ExternalInput")
    aid = nc.dram_tensor("ainv", [2 * NMODE, 1024], bf16, kind="ExternalInput")
    w1cad = nc.dram_tensor("w1ca", [96, 768], bf16, kind="ExternalInput")
    w1cbd = nc.dram_tensor("w1cb", [96, 768], bf16, kind="ExternalInput")
    b1cd = nc.dram_tensor("b1c", [96, 8], f32, kind="ExternalInput")
    dcvd = nc.dram_tensor("dcv", [96, 8], f32, kind="ExternalInput")
    w2cd = nc.dram_tensor("w2c", [97, 768], bf16, kind="ExternalInput")
    ln1gd = nc.dram_tensor("ln1g", [128, DIM], bf16, kind="ExternalInput")
    ln1bd = nc.dram_tensor("ln1b", [128, DIM], bf16, kind="ExternalInput")
    w1fd = nc.dram_tensor("w1f", [DIM, HID], bf16, kind="ExternalInput")
    b1fvd = nc.dram_tensor("b1fv", [128, 12], f32, kind="ExternalInput")
    w2fd = nc.dram_tensor("w2f", [HID, DIM], bf16, kind="ExternalInput")
    b2fd = nc.dram_tensor("b2f", [1, DIM], bf16, kind="ExternalInput")
    w0vd = nc.dram_tensor("w0v", [128, 12], f32, kind="ExternalInput")
    dwbvd = nc.dram_tensor("dwbv", [128, 12], f32, kind="ExternalInput")
    dvevd = nc.dram_tensor("dvev", [128, 48], f32, kind="ExternalInput")
    ddiagd = nc.dram_tensor("ddiag", [12, 128, 512], bf16, kind="ExternalInput")

    def shifted(ap2d, elem_off, length):
        return bass.AP(tensor=ap2d.tensor, offset=ap2d.offset + elem_off,
                       ap=[ap2d.ap[0], [1, length]])

    with tile.TileContext(nc) as tc, \
         ExitStack() as ctx:
        dma = nc.sync.dma_start

        keep = ctx.enter_context(tc.tile_pool(name="keep", bufs=1))
        stat = ctx.enter_context(tc.tile_pool(name="stat", bufs=8))
        x2_pool = ctx.enter_context(tc.tile_pool(name="x2p", bufs=1))
        eps_t = keep.tile([128, 1], f32, tag="eps")
        nc.vector.memset(eps_t[:, :], EPS)
        ident = keep.tile([128, 128], bf16, tag="ident")
        make_identity(nc, ident[:, :])
        ident_h = keep.tile([128, 128], f16, tag="identh")
        make_identity(nc, ident_h[:, :])

        # ---- weight prefetch (overlaps with LN1/DFT) ----
        ai_pool = ctx.enter_context(tc.tile_pool(name="aip", bufs=1))
        av = []
        aoff = 0
        for q in range(10):
            rows = IDFT_CHUNKS[q // 2]
            t = ai_pool.tile([128, 1024], bf16, tag=f"av{q}")
            dma(out=t[:rows, :], in_=aid.ap()[aoff: aoff + rows, :])
            av.append(t)
            aoff += rows
        afw = ctx.enter_context(tc.tile_pool(name="afw", bufs=1))
        w1ca = afw.tile([96, 768], bf16, tag="w1ca")
        w1cb = afw.tile([96, 768], bf16, tag="w1cb")
        w2c = afw.tile([97, 768], bf16, tag="w2c")
        b1c = afw.tile([96, 8], f32, tag="b1c")
        dcv = afw.tile([96, 8], f32, tag="dcv")
        dma(out=w1ca[:, :], in_=w1cad.ap())
        dma(out=w1cb[:, :], in_=w1cbd.ap())
        dma(out=w2c[:, :], in_=w2cd.ap())
        dma(out=b1c[:, :], in_=b1cd.ap())
        dma(out=dcv[:, :], in_=dcvd.ap())

        def layer_stats(xs, tagsuf):
            st = stat.tile([128, 6], f32, tag="st" + tagsuf)
            nc.vector.bn_stats(out=st[:, :], in_=xs)
            mv = stat.tile([128, 2], f32, tag="mv" + tagsuf)
            nc.vector.bn_aggr(out=mv[:, :], in_=st[:, :])
            rstd = stat.tile([128, 1], f32, tag="rs" + tagsuf)
            nc.scalar.activation(out=rstd[:, :], in_=mv[:, 1:2],
                                 func=ACTF.Sqrt, bias=eps_t[:, :])
            nc.vector.reciprocal(out=rstd[:, :], in_=rstd[:, :])
            return mv, rstd

        # ---------------- Phase A: load x, LN1 (b-major) ----------------
        x_pool = tc.alloc_tile_pool(name="xp", bufs=1)
        z1_pool = tc.alloc_tile_pool(name="z1p", bufs=1)
        n1_pool = tc.alloc_tile_pool(name="n1p", bufs=1)

        if not ln1_trivial:
            g_bc = n1_pool.tile([128, DIM], bf16, tag="g_bc")
            b_bc = n1_pool.tile([128, DIM], bf16, tag="b_bc")
            dma(out=g_bc[:, :], in_=ln1gd.ap())
            dma(out=b_bc[:, :], in_=ln1bd.ap())

        xr = xd.ap().rearrange("b (n p) c -> n p b c", p=128)
        x_t, z1_t, n1_t = [], [], []
        with nc.named_scope("ln1"):
            for i in range(8):
                xt = x_pool.tile([128, B_LOC, DIM], f32, tag=f"x{i}")
                dma(out=xt[:, :, :], in_=xr[i])
                x_t.append(xt)
                z1 = z1_pool.tile([128, B_LOC, DIM], bf16, tag=f"z1_{i}")
                z1_t.append(z1)
                if ln1_trivial:
                    n1_t.append(z1)  # n1 == z1 when gamma=1, beta=0
                else:
                    n1_t.append(n1_pool.tile([128, B_LOC, DIM], bf16,
                                             tag=f"n1_{i}"))
            for b in range(B_LOC):
                for i in range(8):
                    xs = x_t[i][:, b, :]
                    mv, rstd = layer_stats(xs, "1")
                    nc.vector.tensor_scalar(out=z1_t[i][:, b, :], in0=xs,
                                            scalar1=mv[:, 0:1],
                                            scalar2=rstd[:, :],
                                            op0=ALU.subtract, op1=ALU.mult)
                    if not ln1_trivial:
                        nc.vector.tensor_mul(out=n1_t[i][:, b, :],
                                             in0=z1_t[i][:, b, :],
                                             in1=g_bc[:, :])
                        nc.vector.tensor_add(out=n1_t[i][:, b, :],
                                             in0=n1_t[i][:, b, :],
                                             in1=b_bc[:, :])

        # ---------------- Phase B: forward DFT ----------------
        xf_pool = tc.alloc_tile_pool(name="xfp", bufs=1)
        xf_t = []
        with nc.named_scope("dft"):
            with tc.tile_pool(name="tfp", bufs=1) as tf_pool, \
                 tc.tile_pool(name="dftps", bufs=2, space="PSUM") as dft_ps:
                tft = []
                for k in range(8):
                    t = tf_pool.tile([128, 2 * NMODE], bf16, tag=f"tf{k}")
                    dma(out=t[:, :], in_=tfd.ap()[128 * k: 128 * (k + 1), :])
                    tft.append(t)
                for mt in range(8):
                    b, bpair = divmod(mt, 4)
                    xf = xf_pool.tile([96, 2 * NMODE], bf16, tag=f"xf{mt}")
                    for half in range(2):
                        c0 = NMODE * half
                        ps = dft_ps.tile([96, 544], f32, tag="dftps")
                        for w0, wlen in ((0, 512), (512, 32)):
                            for k in range(8):
                                nc.tensor.matmul(
                                    ps[:, w0: w0 + wlen],
                                    z1_t[k][:, b, 96 * bpair: 96 * bpair + 96],
                                    tft[k][:, c0 + w0: c0 + w0 + wlen],
                                    start=(k == 0), stop=(k == 7))
                        nc.scalar.copy(out=xf[:, c0: c0 + NMODE],
                                       in_=ps[:, :])
                    xf_t.append(xf)

        # ---------------- Phase C: frequency-domain block MLP ----------------
        o2s_pool = tc.alloc_tile_pool(name="o2sp", bufs=1)
        o2s_t = [o2s_pool.tile([IDFT_CHUNKS[t], B_LOC, DIM, 2], bf16,
                               tag=f"o2s{t}", name=f"o2s{t}") for t in range(5)]
        with nc.named_scope("blockmlp"):
            with tc.tile_pool(name="o1p", bufs=8) as o1_pool, \
                 tc.tile_pool(name="l1ps", bufs=2, space="PSUM") as l1_ps, \
                 tc.tile_pool(name="l2ps", bufs=2, space="PSUM") as l2_ps:
                for b in range(B_LOC):
                    o1_b = []
                    for n in range(NUM_BLOCKS):
                        xf = xf_t[b * 4 + n // 2]
                        o1 = o1_pool.tile([97, NMODE], bf16, tag="o1")
                        nc.gpsimd.memset(o1[96:97, :], 1.0)
                        ps = l1_ps.tile([96, 544], f32, tag="l1ps")
                        for w0, wlen in ((0, 512), (512, 32)):
                            nc.tensor.matmul(
                                ps[:, w0: w0 + wlen],
                                w1ca[:, 96 * n: 96 * (n + 1)],
                                xf[:, w0: w0 + wlen],
                                start=True, stop=False)
                            nc.tensor.matmul(
                                ps[:, w0: w0 + wlen],
                                w1cb[:, 96 * n: 96 * (n + 1)],
                                xf[:, NMODE + w0: NMODE + w0 + wlen],
                                start=False, stop=True)
                        nc.vector.tensor_add(out=ps[:, 0:1], in0=ps[:, 0:1],
                                             in1=dcv[:, n: n + 1])
                        nc.scalar.activation(out=o1[0:96, :], in_=ps[:, :],
                                             func=ACTF.Relu,
                                             bias=b1c[:, n: n + 1])
                        o1_b.append(o1)
                    for mc in range(5):
                        rows = IDFT_CHUNKS[mc]
                        for hn in range(2):
                            ps2 = l2_ps.tile([128, 384], f32, tag="l2ps")
                            for j in range(4):
                                n = hn * 4 + j
                                nc.tensor.matmul(
                                    ps2[:rows, 96 * j: 96 * (j + 1)],
                                    o1_b[n][:, 128 * mc: 128 * mc + rows],
                                    w2c[:, 96 * n: 96 * (n + 1)],
                                    start=True, stop=True)
                            nc.scalar.copy(
                                out=o2s_t[mc][:rows, b,
                                              hn * 192: (hn + 1) * 192, :]
                                    .rearrange("p c r -> p (c r)"),
                                in_=ps2[:rows, :])

        # softshrink in place
        with nc.named_scope("softshrink"):
            with tc.tile_pool(name="sshp", bufs=2) as ssh_pool:
                for t in range(5):
                    rows = IDFT_CHUNKS[t]
                    flat = o2s_t[t][:, :, :, :].rearrange("p b c r -> p (b c r)")
                    tmp = ssh_pool.tile([128, B_LOC * DIM * 2], bf16, tag="ssh")
                    nc.vector.tensor_scalar(out=tmp[:rows, :], in0=flat,
                                            scalar1=-LAM, scalar2=LAM,
                                            op0=ALU.max, op1=ALU.min)
                    nc.vector.scalar_tensor_tensor(
                        out=flat, in0=tmp[:rows, :], scalar=-1.0, in1=flat,
                        op0=ALU.mult, op1=ALU.add)

        # ---------------- Phase D: inverse DFT + residual ----------------
        x2_t = []
        with nc.named_scope("idft"):
            with tc.tile_pool(name="idftps", bufs=4, space="PSUM") as idft_ps:
                for mt in range(8):
                    x2 = x2_pool.tile([128, B_LOC, DIM], bf16, tag=f"x2_{mt}")
                    for b in range(B_LOC):
                        ps = idft_ps.tile([128, DIM], f32, tag="idftps")
                        for q in range(10):
                            rows = IDFT_CHUNKS[q // 2]
                            nc.tensor.matmul(
                                ps[:, :],
                                av[q][:rows, 128 * mt: 128 * (mt + 1)],
                                o2s_t[q // 2][:rows, b, :, q % 2],
                                start=(q == 0), stop=False)
                        # + n1 residual (z1 when LN1 is trivial)
                        nc.tensor.matmul(
                            ps[:, :], ident[:, :], n1_t[mt][:, b, :],
                            start=False, stop=True)
                        nc.vector.tensor_add(out=x2[:, b, :],
                                             in0=x_t[mt][:, b, :], in1=ps[:, :])
                    x2_t.append(x2)
        o2s_pool.release()
        xf_pool.release()
        n1_pool.release()
        z1_pool.release()
        x_pool.release()

        # ---------------- Phase E: MLP ----------------
        mlpw = ctx.enter_context(tc.tile_pool(name="mlpw", bufs=1))
        w1ft = []
        for k in range(3):
            t = mlpw.tile([128, HID], bf16, tag=f"w1f{k}")
            dma(out=t[:, :], in_=w1fd.ap()[128 * k: 128 * (k + 1), :])
            w1ft.append(t)
        w2ft = []
        for k in range(12):
            t = mlpw.tile([128, DIM], bf16, tag=f"w2f{k}")
            dma(out=t[:, :], in_=w2fd.ap()[128 * k: 128 * (k + 1), :])
            w2ft.append(t)
        b2f = mlpw.tile([1, DIM], bf16, tag="b2f")
        dma(out=b2f[:, :], in_=b2fd.ap())
        ones_b = mlpw.tile([1, 1024], bf16, tag="ones")
        nc.vector.memset(ones_b[:, :], 1.0)
        b1fv = mlpw.tile([128, 12], f32, tag="b1fv")
        w0v = mlpw.tile([128, 12], f32, tag="w0v")
        dwbv = mlpw.tile([128, 12], f32, tag="dwbv")
        dvev = mlpw.tile([128, 48], f32, tag="dvev")
        dma(out=b1fv[:, :], in_=b1fvd.ap())
        dma(out=w0v[:, :], in_=w0vd.ap())
        dma(out=dwbv[:, :], in_=dwbvd.ap())
        dma(out=dvev[:, :], in_=dvevd.ap())
        ddiag_t = []
        for t in range(12):
            dt_ = mlpw.tile([128, 512], bf16, tag=f"dd{t}")
            dma(out=dt_[:, :], in_=ddiagd.ap()[t])
            ddiag_t.append(dt_)
        hp_t = []
        y_t = []
        for t in range(12):
            hp = mlpw.tile([128, HP_LEN], bf16, tag=f"hp{t}")
            nc.gpsimd.memset(hp[:, :], 0.0)
            hp_t.append(hp)
            y_t.append(mlpw.tile([128, 1024], bf16, tag=f"y{t}", name=f"y{t}"))
        z2T_t = [mlpw.tile([128, 1024], bf16, tag=f"z2T{c}", name=f"z2T{c}")
                 for c in range(3)]

        with tc.tile_pool(name="z2p", bufs=3) as z2_pool, \
             tc.tile_pool(name="accp", bufs=3) as acc_pool, \
             tc.tile_pool(name="outp", bufs=3) as out_pool, \
             tc.tile_pool(name="mlpps", bufs=2, space="PSUM") as mlp_ps, \
             tc.tile_pool(name="cvps", bufs=2, space="PSUM") as cv_ps:
            for b in range(B_LOC):
                with nc.named_scope("ln2t"):
                    for i in range(8):
                        xs = x2_t[i][:, b, :]
                        z2 = z2_pool.tile([128, DIM], bf16, tag="z2")
                        mv, rstd = layer_stats(xs, "2")
                        nc.vector.tensor_scalar(out=z2[:, :], in0=xs,
                                                scalar1=mv[:, 0:1],
                                                scalar2=rstd[:, :],
                                                op0=ALU.subtract, op1=ALU.mult)
                        for c in range(3):
                            pst = mlp_ps.tile([128, 128], bf16, tag="trps")
                            nc.tensor.transpose(
                                pst[:, :], z2[:, 128 * c: 128 * (c + 1)],
                                ident[:, :])
                            nc.scalar.copy(
                                out=z2T_t[c][:, 128 * i: 128 * (i + 1)],
                                in_=pst[:, :])
                with nc.named_scope("fc1"):
                    for t in range(12):
                        for ncb in range(2):
                            ps = mlp_ps.tile([128, 512], f32, tag="fc1ps")
                            for k in range(3):
                                nc.tensor.matmul(
                                    ps[:, :],
                                    w1ft[k][:, 128 * t: 128 * (t + 1)],
                                    z2T_t[k][:, 512 * ncb: 512 * (ncb + 1)],
                                    start=(k == 0), stop=(k == 2))
                            hp2d = hp_t[t][:, :]
                            dest = bass.AP(
                                tensor=hp2d.tensor,
                                offset=hp2d.offset + 36 + 544 * ncb,
                                ap=[hp2d.ap[0], [34, 16], [1, 32]])
                            nc.scalar.activation(
                                out=dest,
                                in_=ps[:, :].rearrange("p (h w) -> p h w", w=32),
                                func=ACTF.Identity, bias=b1fv[:, t: t + 1])
                with nc.named_scope("conv"):
                    for t in range(12):
                        hp2d = hp_t[t][:, :]
                        # fp16 SBUF accumulator: center + 2 taps on DVE,
                        # 2 taps on GpSimd
                        acc = acc_pool.tile([128, CONV_LEN], f16, tag="acc")
                        nc.vector.tensor_scalar(
                            out=acc[:, :], in0=shifted(hp2d, 35, CONV_LEN),
                            scalar1=w0v[:, t: t + 1],
                            scalar2=dwbv[:, t: t + 1],
                            op0=ALU.mult, op1=ALU.add)
                        for j, (dy, dx) in enumerate(TAPS_DVE):
                            dd = dy * PADW + dx
                            nc.vector.scalar_tensor_tensor(
                                out=acc[:, :],
                                in0=shifted(hp2d, 35 + dd, CONV_LEN),
                                scalar=dvev[:, 12 * j + t: 12 * j + t + 1],
                                in1=acc[:, :], op0=ALU.mult, op1=ALU.add)
                        for hf in range(2):
                            hoff = CONV_HALF * hf
                            cps = cv_ps.tile([128, CONV_HALF], f32, tag="cvps")
                            for w0, wlen in ((0, 512), (512, 32)):
                                for j, (dy, dx) in enumerate(TAPS_PE):
                                    dd = dy * PADW + dx
                                    nc.tensor.matmul(
                                        cps[:, w0: w0 + wlen],
                                        ddiag_t[t][:, 128 * j: 128 * (j + 1)],
                                        shifted(hp2d, 35 + hoff + w0 + dd, wlen),
                                        start=(j == 0), stop=False,
                                        skip_group_check=True)
                                nc.tensor.matmul(
                                    cps[:, w0: w0 + wlen],
                                    ident_h[:, :],
                                    shifted(acc[:, :], hoff + w0, wlen),
                                    start=False, stop=True,
                                    skip_group_check=True)
                            cps2d = cps[:, :]
                            nc.scalar.activation(
                                out=y_t[t][:, 512 * hf: 512 * (hf + 1)]
                                    .rearrange("p (h w) -> p h w", w=32),
                                in_=bass.AP(tensor=cps2d.tensor,
                                            offset=cps2d.offset + 1,
                                            ap=[cps2d.ap[0], [34, 16], [1, 32]]),
                                func=ACTF.Gelu)
                with nc.named_scope("fc2"):
                    for mc in range(8):
                        psf = cv_ps.tile([128, CONV_HALF], f32, tag="cvps")
                        ps = psf[:, :DIM]
                        for k in range(12):
                            nc.tensor.matmul(
                                ps[:, :], y_t[k][:, 128 * mc: 128 * (mc + 1)],
                                w2ft[k][:, :], start=(k == 0), stop=False)
                        nc.tensor.matmul(
                            ps[:, :], ones_b[:, 128 * mc: 128 * (mc + 1)],
                            b2f[:, :], start=False, stop=False)
                        # + x2 residual
                        nc.tensor.matmul(
                            ps[:, :], ident[:, :], x2_t[mc][:, b, :],
                            start=False, stop=True)
                        ot = out_pool.tile([128, DIM], f32, tag="out")
                        nc.scalar.copy(out=ot[:, :], in_=ps[:, :])
                        dma(out=yd.ap()[b, 128 * mc: 128 * (mc + 1), :],
                            in_=ot[:, :])

    return nc


_NC_CACHE = {}


def kernel(**inputs):
    from concourse.bass_utils import run_bass_kernel_spmd

    x = np.ascontiguousarray(np.asarray(inputs["x"], np.float32))
    assert int(inputs["H"]) == H and int(inputs["W"]) == W
    der = host_derived(inputs)
    trivial = (np.allclose(np.asarray(inputs["ln1_g"]), 1.0)
               and np.allclose(np.asarray(inputs["ln1_b"]), 0.0))

    if trivial not in _NC_CACHE:
        nc = build_nc(ln1_trivial=trivial)
        nc.compile()
        _NC_CACHE[trivial] = nc
    nc = _NC_CACHE[trivial]

    in_maps = []
    for c in range(N_CORES):
        m = dict(der)
        m["x"] = np.ascontiguousarray(x[c * B_LOC: (c + 1) * B_LOC])
        in_maps.append(m)
    res = run_bass_kernel_spmd(nc, in_maps, core_ids=list(range(N_CORES)))
    out = np.concatenate([res.results[c]["y"] for c in range(N_CORES)], axis=0)
    return out.astype(np.float32)
